# revision 45
# baseline (speedup 1.0000x reference)
"""Fast-weight-sum (causal linear attention) transformer layer on 8 TRN2 cores.

Sharding: data-parallel over batch — BSZ=8 batches, one per NeuronCore, no
collectives. Each core runs the full layer for its batch column of h.

Per-core algorithm (L=1024, D=512, H=8 heads, dh=64, chunk C=128):
  qkv = h @ W_qkv in fp8(e4m3) (halves the input DMA; feature map +
        normalizations wash the fp8 noise to ~4e-4 in the final output).
        W_qkv pre-permuted on host to g-major [Q|K|V] blocks (contiguous
        ramp DMAs); the V block is pre-scaled by 1/sqrt(dh).
  q <- elu(q)+1 (not normalized: EPS*sum_d(q) is tracked via the
       constant-1 column of s_ext); k stays raw and 1/sum_d(k) is folded
       into v_ext's value and denominator columns per row.
  Chunked causal linear attention, chunk-parallel:
    A^T[s,t] = k_s.q_t for 8 heads packed into 2 PSUM banks, masked with
    2 batched DVE multiplies per chunk.
    skv_c    = k_c^T @ [v|krec|0] per head pair; only the per-head
               DIAGONAL blocks are evacuated (ACT), so the prefix state
               stays block-diagonal and one dense gpsimd bf16 add per
               chunk maintains s_ext_c = s_ext_{c-1} + skv_{c-1}.
    out      = A^T.T @ [v|krec|0]  +  q @ [S|kstate|1]; the q@S matmul
               runs FUSED over the full 128-partition head pair (the
               block-diagonal state makes the cross terms zero).
  attn_h = out_h / out_h[:,64]  (den column; the reference's +EPS term is
          <=6.4e-4 relative — dropped), applied as one batched DVE
          multiply per PSUM bank with the broadcast reciprocal.
  layer_out = attn @ W_o (bf16) with the residual h accumulated on PE via
  an identity matmul into the same PSUM bank; layernorm stats and the
  normalize (ACT Identity with bias/scale) read straight from PSUM.
  Output is stored bf16 and upcast to fp32 on host.

Emission is two dense phases: phase A = all qkv + transposes; phase B =
per chunk [state, chunk, out-projection(c-1)]. Measured and REJECTED on
hardware: fp8 DoubleRow qkv (96->48 matmuls starves the PE HAM window —
whole kernel drops to 1.2 GHz, 92->99us), phase interleaving, hoisting
states into phase A (79->98us), masks/adds via ACT-copy+gpsimd (stalls
the A@v matmuls ~900ns/chunk), ppb bufs 4->3. The dense N=512 qkv
stream is what holds the PE clock at 2.4 GHz.

elu(x)+1 = min(exp(x),1)+relu(x) is one ACT Exp + one custom DVE op
(ELU1_ANT, registered below) instead of Exp/Relu/combine.
"""

import numpy as np

import concourse.bass as bass
import concourse.mybir as mybir
import concourse.tile as tile
from concourse import bacc
from concourse import dve_ops as _dvo
from concourse.bass_utils import run_bass_kernel_spmd


def _register_elu1():
    """Custom DVE op: out = min(in0, s0) + relu(in1), so the elu(x)+1
    feature map is one ACT Exp + one DVE op (no separate relu pass)."""
    name = "ELU1_ANT"
    if name not in _dvo._SUB_OPCODE_FOR_NAME:
        from concourse.dve_spec import (Spec, Src0, Src1, C0, minn, relu,
                                        lower, _has_src1)
        from concourse.dve_uop import DveOpSpec

        spec = Spec(
            body=minn(Src0, C0) + relu(Src1),
            reference=lambda in0, in1, s0, s1, imm2: (
                np.minimum(in0.astype(np.float32), s0)
                + np.maximum(np.nan_to_num(in1.astype(np.float32), nan=0.0),
                             0.0)),
        )
        opcode = _dvo._CUSTOM_DVE_ROW_BASE + len(_dvo.OPS)
        _dvo._SUB_OPCODE_FOR_NAME[name] = opcode
        shas = {}
        for ver in ("v3", "v4"):
            try:
                s = DveOpSpec(name=name, opcode=opcode,
                              uops=lower(spec, ver=ver),
                              rd1_en=_has_src1(spec))
                shas[ver] = s.sha(ver)
            except Exception:
                pass
        op = _dvo.DveOp(name, spec, subdim=False, uops_sha=shas)
        _dvo.OPS.append(op)
        _dvo.CUSTOM_DVE_SPECS[name] = spec
    return next(o for o in _dvo.OPS if o.name == name)


_ELU1 = _register_elu1()

L, D, F, H, DH = 1024, 512, 1536, 8, 64
P = 128
NLT = L // P          # 8 l-tiles == chunks
KD = D // P           # 4 contraction tiles of d_model
EW = DH + 2           # 66: per-head extended width [out | den | one]
EPS = 1e-5
LN_EPS = 1e-5
SCALE = 1.0 / np.sqrt(DH)
BF = mybir.dt.bfloat16
F8 = mybir.dt.float8e4
F32 = mybir.dt.float32
AX = mybir.AluOpType
ACTF = mybir.ActivationFunctionType
USE_FP8 = True
WAR_BLOCKERS = True

LAST_RESULT = None


def _build_core_kernel(nc, tc, apply_gb=True):
    QDT = F8 if USE_FP8 else BF
    hT_d = nc.dram_tensor("hT", (P, NLT, KD, P), QDT, kind="ExternalInput")
    # g-major weight layout: per-(g, kt) slices are contiguous per partition,
    # so the ramp DMAs are unstrided and the first matmul starts early.
    wq_d = nc.dram_tensor("W_qkv", (P, 3, KD, D), QDT, kind="ExternalInput")
    h_d = nc.dram_tensor("h", (P, NLT, D), BF, kind="ExternalInput")
    wo_d = nc.dram_tensor("W_o", (P, KD, D), BF, kind="ExternalInput")
    gamma_d = nc.dram_tensor("gamma", (D,), F32, kind="ExternalInput")
    beta_d = nc.dram_tensor("beta", (D,), F32, kind="ExternalInput")
    out_d = nc.dram_tensor("out", (L, D), BF, kind="ExternalOutput")

    with (
        tc.tile_pool(name="consts", bufs=1) as consts,
        tc.tile_pool(name="work", bufs=3) as work,
        tc.tile_pool(name="sext", bufs=3) as sext_pool,
        tc.tile_pool(name="pmm", bufs=2, space="PSUM") as pmm,
        tc.tile_pool(name="pscratch", bufs=2, space="PSUM") as pscratch,
        tc.tile_pool(name="ppb", bufs=4, space="PSUM") as ppb,
    ):
        # ---------- constants on gpsimd (ready before chunk 0) ----------
        utri4 = consts.tile([P, 4, P], BF, tag="utri4")
        nc.gpsimd.memset(utri4, 0.0)
        nc.gpsimd.affine_select(
            out=utri4, in_=utri4, compare_op=AX.is_gt, fill=1.0,
            base=0, pattern=[[0, 4], [-1, P]], channel_multiplier=1,
        )
        # bf16 identity: the out-projection accumulates h via I.T @ h on PE,
        # replacing the DVE residual add.
        ident = consts.tile([P, P], BF, tag="ident")
        nc.gpsimd.memset(ident, 0.0)
        nc.gpsimd.affine_select(
            out=ident, in_=ident, compare_op=AX.is_gt, fill=1.0,
            base=0, pattern=[[-1, P]], channel_multiplier=1,
        )
        nc.gpsimd.affine_select(
            out=ident, in_=ident, compare_op=AX.is_ge, fill=0.0,
            base=0, pattern=[[-1, P]], channel_multiplier=1,
        )
        # v_ext: per (lt, pair p) the 132 columns are [vA|krecA|0|vB|krecB|0]
        v_ext = consts.tile([P, NLT, 4, 2 * EW], BF, tag="v_ext")
        vc = v_ext.rearrange("p l f (j e) -> p l f j e", e=EW)
        nc.gpsimd.memset(vc[:, :, :, :, DH + 1:DH + 2], 0.0)
        # Prefix attention states for all chunks, [P, chunk, pair, 132].
        # Chunk 0's state is zeros. (The reference's +EPS denominator term
        # is <=6.4e-4 relative to the denominator — dropped, so no qsum
        # column is tracked.)
        # States are kept BLOCK-DIAGONAL per pair (head A rows x head A cols,
        # head B rows x head B cols; cross blocks stay zero) so the q@S
        # matmul runs fused over the full 128-partition pair in one shot.
        s_full = consts.tile([P, NLT, 4, 2 * EW], BF, tag="s_full")
        nc.gpsimd.memset(s_full[:, 0], 0.0)
        # skv staging buffer: zeroed once; the per-chunk ACT copies only ever
        # write the diagonal blocks, so the cross blocks stay zero and the
        # dense gpsimd prefix-add preserves block-diagonality. One persistent
        # buffer is enough — the prefix chain is serial regardless.
        skv_sb = consts.tile([P, 4, 2 * EW], BF, tag="skv_sb")
        nc.gpsimd.memset(skv_sb, 0.0)

        eps_sb = consts.tile([P, 1], F32, tag="eps_sb")
        nc.vector.memset(eps_sb, LN_EPS)


        # ---------- input DMAs ----------
        # sync (HWDGE) ring: qkv operands, first-needed first (few, coarse
        # triggers — descriptor-gen occupies the issuing sequencer).
        wq_b = consts.tile([P, 3, KD, D], QDT, tag="wq_b")
        hT = consts.tile([P, NLT, KD, P], QDT, tag="hT")
        nc.scalar.dma_start(hT[:, 0], hT_d[:, 0])
        nc.sync.dma_start(wq_b[:, 0, 0], wq_d[:, 0, 0])
        nc.scalar.dma_start(hT[:, 1], hT_d[:, 1])
        nc.sync.dma_start(wq_b[:, 0, 1:4], wq_d[:, 0, 1:4])
        nc.scalar.dma_start(hT[:, 2:4], hT_d[:, 2:4])
        nc.sync.dma_start(wq_b[:, 1], wq_d[:, 1])
        nc.scalar.dma_start(hT[:, 4:8], hT_d[:, 4:8])
        nc.sync.dma_start(wq_b[:, 2], wq_d[:, 2])
        # gpsimd (SWDGE) ring: late consumers (wo/h are needed only from
        # the out-projection ~40us in). A WAR blocker makes their DMAs wait
        # until qkv l-tile 1 is through, so the ramp-critical hT/wq loads
        # get the full HBM bandwidth.
        h_bf = consts.tile([P, NLT, D], BF, tag="h_bf")
        wo_b = consts.tile([P, KD, D], BF, tag="wo_b")
        blk = consts.tile([P, 1], F32, tag="blk")
        if apply_gb:
            gamma_ap = gamma_d[:]
            gamma_bc = consts.tile([P, D], BF, tag="gamma_bc")
            nc.gpsimd.dma_start(
                gamma_bc,
                bass.AP(tensor=gamma_ap.tensor, offset=gamma_ap.offset,
                        ap=[[0, P]] + list(gamma_ap.ap)),
            )
            beta_ap = beta_d[:]
            beta_bc = consts.tile([P, D], F32, tag="beta_bc")
            nc.gpsimd.dma_start(
                beta_bc,
                bass.AP(tensor=beta_ap.tensor, offset=beta_ap.offset,
                        ap=[[0, P]] + list(beta_ap.ap)),
            )

        qk_sb = consts.tile([P, NLT, 2 * D], BF, tag="qk_sb")
        qkT = consts.tile([P, NLT, 8, P], BF, tag="qkT")
        attn = consts.tile([P, NLT, D], BF, tag="attn")
        attnT = consts.tile([P, NLT, KD, P], BF, tag="attnT")

        # WAR blockers: reading wo_b/h_bf with a dependency on qkv(lt=1)
        # forces their bulk DMAs to start only after the ramp. (Disabled in
        # CoreSim runs: the intentional garbage read trips its init checker.)
        if WAR_BLOCKERS:
            nc.gpsimd.tensor_tensor(blk, wo_b[:, 0, 0:1], qk_sb[:, 1, 0:1],
                                    AX.add)
            nc.gpsimd.tensor_tensor(blk, h_bf[:, 0, 0:1], qk_sb[:, 1, 0:1],
                                    AX.add)
        nc.gpsimd.dma_start(wo_b, wo_d[:])
        nc.gpsimd.dma_start(h_bf[:, 0:4], h_d[:, 0:4])
        nc.gpsimd.dma_start(h_bf[:, 4:8], h_d[:, 4:8])

        def emit_qkv(lt):
            for g in range(3):  # 0=q, 1=k, 2=v
                pm = pmm.tile([P, D], F32, tag="mm")
                for kt in range(KD):
                    nc.tensor.matmul(
                        pm,
                        lhsT=hT[:, lt, kt, :],
                        rhs=wq_b[:, g, kt],
                        start=(kt == 0),
                        stop=(kt == KD - 1),
                    )
                if g == 2:
                    # v_ext value cols: v * krec (row-normalizes k's effect);
                    # krec col carries the denominator contribution.
                    nc.vector.tensor_tensor(
                        vc[:, lt, :, :, 0:DH],
                        pm.rearrange("p (f j e) -> p f j e", f=4, j=2),
                        krec[:, :, None].rearrange(
                            "p (f j) x -> p f j x", j=2).to_broadcast(
                                (P, 4, 2, DH)),
                        AX.mult,
                    )
                    nc.scalar.copy(
                        vc[:, lt, :, :, DH:DH + 1],
                        krec.rearrange("p (f j) -> p f j", j=2)[:, :, :, None])
                else:
                    # elu(x)+1 == min(exp(x), 1) + relu(x)
                    e1 = work.tile([P, D], BF, tag="fmap_e")
                    nc.scalar.activation(e1, pm, ACTF.Exp)
                    if g == 0:
                        nc.vector._custom_dve(
                            _ELU1, out=qk_sb[:, lt, 0:D], in0=e1, in1=pm,
                            s0=1.0)
                    else:
                        nc.vector._custom_dve(
                            _ELU1, out=qk_sb[:, lt, D:2 * D], in0=e1, in1=pm,
                            s0=1.0)
                        ksum = work.tile([P, H], F32, tag="ksum")
                        nc.vector.reduce_sum(
                            out=ksum,
                            in_=qk_sb[:, lt, D:2 * D].rearrange(
                                "p (h e) -> p h e", e=DH),
                            axis=mybir.AxisListType.X,
                        )
                        krec = work.tile([P, H], F32, tag="krec")
                        nc.vector.reciprocal(krec, ksum)
            nc.sync.dma_start_transpose(qkT[:, lt], qk_sb[:, lt])

        def emit_state(lt):
            # chunk-local state + prefix add (phase-B prelude; all inputs
            # ready, so these run dense and the chain resolves quickly):
            # skv[f, :] = sum_s k[s,f] * [v|krec|0]
            skv = [pscratch.tile([P, 2, 2 * EW], F32, tag="scr",
                                 name=f"skv{i}") for i in range(2)]
            for p in range(4):
                nc.tensor.matmul(
                    skv[p // 2][:, p % 2],
                    lhsT=qk_sb[:, lt, D + p * P:D + (p + 1) * P],
                    rhs=v_ext[:, lt, p], start=True, stop=True)
            # Evacuate only the diagonal blocks (ACT), then one dense bf16
            # prefix-add on gpsimd — keeps DVE free and state block-diagonal.
            for i in range(2):
                nc.scalar.copy(skv_sb[0:DH, 2 * i:2 * i + 2, 0:EW],
                               skv[i][0:DH, :, 0:EW])
                nc.scalar.copy(skv_sb[DH:P, 2 * i:2 * i + 2, EW:2 * EW],
                               skv[i][DH:P, :, EW:2 * EW])
            nc.gpsimd.tensor_tensor(s_full[:, lt + 1], s_full[:, lt],
                                    skv_sb, AX.add)

        def emit_ab(c):
            # A^T for 8 heads into 2 banks; head A rows (contraction
            # partitions 0:64) -> bank X, head B -> bank Y (concurrent).
            abX = pscratch.tile([P, 4, P], F32, tag="scr", name="abX")
            abY = pscratch.tile([P, 4, P], F32, tag="scr", name="abY")
            for p in range(4):
                nc.tensor.matmul(abX[:, p], lhsT=qkT[0:DH, c, 4 + p, :],
                                 rhs=qkT[0:DH, c, p, :], start=True, stop=True)
                nc.tensor.matmul(abY[:, p], lhsT=qkT[DH:P, c, 4 + p, :],
                                 rhs=qkT[DH:P, c, p, :], start=True, stop=True)
            # Mask-multiply directly off PSUM on DVE: the ACT-copy+gpsimd
            # variant was tried and loses — it puts ~1.5us on the per-chunk
            # critical chain and stalls the A@v matmuls ~900ns every chunk.
            amX = work.tile([P, 4, P], BF, tag="amX")
            amY = work.tile([P, 4, P], BF, tag="amY")
            nc.vector.tensor_tensor(amX, abX, utri4, AX.mult)
            nc.vector.tensor_tensor(amY, abY, utri4, AX.mult)
            return amX, amY

        def emit_chunk(c, am=None):
            amX, amY = am if am is not None else emit_ab(c)
            pbs = [ppb.tile([P, 2, 2 * EW], F32, tag="pb", name=f"pb{i}")
                   for i in range(2)]
            sc = s_full[:, c]
            for i in range(2):
                for j in range(2):
                    p = 2 * i + j
                    pb = pbs[i][:, j]
                    # q@S fused across the head pair: the state is
                    # block-diagonal, so one K=128 matmul covers both heads.
                    nc.tensor.matmul(pb, lhsT=qkT[:, c, p, :],
                                     rhs=sc[:, p], start=(j == 0), stop=False)
                    nc.tensor.matmul(pb[:, 0:EW], lhsT=amX[:, p],
                                     rhs=v_ext[:, c, p, 0:EW],
                                     start=False, stop=False)
                    nc.tensor.matmul(pb[:, EW:2 * EW], lhsT=amY[:, p],
                                     rhs=v_ext[:, c, p, EW:2 * EW],
                                     start=False, stop=(j == 1))

            # denominators: reciprocal straight off the PSUM den column,
            # then one batched den-scale DVE multiply per bank.
            denr = work.tile([P, H], F32, tag="denr")
            for i in range(2):
                pbr = pbs[i].rearrange("p f (j e) -> p f j e", e=EW)
                nc.vector.reciprocal(
                    denr[:, 4 * i:4 * i + 4].rearrange("p (f j) -> p f j",
                                                       j=2),
                    pbr[:, :, :, DH])
            ac = attn[:, c].rearrange("p (f e) -> p f e", e=DH)
            for i in range(2):
                pbr = pbs[i].rearrange("p f (j e) -> p f j e", e=EW)
                nc.vector.tensor_tensor(
                    ac[:, 4 * i:4 * i + 4, :],
                    pbr[:, :, :, 0:DH],
                    denr[:, 4 * i:4 * i + 4, None].to_broadcast(
                        (P, 4, DH)),
                    AX.mult,
                )
            nc.sync.dma_start_transpose(attnT[:, c], attn[:, c])

        def emit_outproj(lt):
            pm = pmm.tile([P, D], F32, tag="mm")
            # residual: x = h + attn @ W_o, with h accumulated on PE via I.T@h
            nc.tensor.matmul(pm, lhsT=ident, rhs=h_bf[:, lt], start=True,
                             stop=False)
            for kt in range(KD):
                nc.tensor.matmul(pm, lhsT=attnT[:, lt, kt, :],
                                 rhs=wo_b[:, kt], start=False,
                                 stop=(kt == KD - 1))
            stats = work.tile([P, nc.vector.BN_STATS_DIM], F32, tag="stats")
            nc.vector.bn_stats(out=stats, in_=pm)
            mv = work.tile([P, nc.vector.BN_AGGR_DIM], F32, tag="mv")
            nc.vector.bn_aggr(out=mv, in_=stats)
            std = work.tile([P, 1], F32, tag="std")
            nc.scalar.activation(std, mv[:, 1:2], ACTF.Sqrt, bias=eps_sb,
                                 scale=1.0)
            rstd = work.tile([P, 1], F32, tag="rstd")
            nc.vector.reciprocal(rstd, std)
            nmr = work.tile([P, 1], F32, tag="nmr")
            nc.vector.tensor_scalar(out=nmr, in0=mv[:, 0:1], scalar1=-1.0,
                                    scalar2=rstd, op0=AX.mult, op1=AX.mult)
            xn = work.tile([P, D], BF, tag="xn")
            nc.scalar.activation(xn, pm, ACTF.Identity, bias=nmr, scale=rstd)
            if apply_gb:
                xg = work.tile([P, D], BF, tag="xg")
                nc.vector.tensor_tensor(xg, xn, gamma_bc, AX.mult)
                yo = work.tile([P, D], BF, tag="yo")
                nc.vector.tensor_tensor(yo, xg, beta_bc, AX.add)
                nc.sync.dma_start(out_d[lt * P:(lt + 1) * P, :], yo)
            else:
                nc.sync.dma_start(out_d[lt * P:(lt + 1) * P, :], xn)

        # ---------- phase A: dense qkv (keeps the PE HAM clock warm) ----
        # (Measured: fp8 DoubleRow halves this stream but the PE then never
        # sustains a busy HAM window — the whole kernel runs at 1.2 GHz and
        # loses ~7us net. The dense 96-matmul stream IS the warmth engine.
        # Also measured: hoisting emit_state into this loop regresses 79->98us
        # — the extra ACT/gpsimd traffic here starves the qkv pipeline.)
        for lt in range(NLT):
            emit_qkv(lt)
        # ---------- phase B: attention chunks + out-projection ----------
        for c in range(NLT):
            if c < NLT - 1:
                emit_state(c)
            emit_chunk(c)
            if c >= 1:
                emit_outproj(c - 1)
        emit_outproj(NLT - 1)


_NC_CACHE = {}


def _get_nc(apply_gb=True):
    key = ("nc", apply_gb, WAR_BLOCKERS)
    if key not in _NC_CACHE:
        nc = bacc.Bacc("TRN2", target_bir_lowering=False, debug=False)
        with tile.TileContext(nc) as tc:
            _build_core_kernel(nc, tc, apply_gb=apply_gb)
        nc.compile()
        _NC_CACHE[key] = nc
    return _NC_CACHE[key]


def kernel(h, W_qkv, W_o, gamma, beta, trace=False):
    global LAST_RESULT
    h = np.asarray(h, dtype=np.float32)
    W_qkv = np.asarray(W_qkv, dtype=np.float32)
    W_o = np.asarray(W_o, dtype=np.float32)
    gamma = np.asarray(gamma, dtype=np.float32)
    beta = np.asarray(beta, dtype=np.float32)

    import ml_dtypes
    bf16 = ml_dtypes.bfloat16
    f8 = ml_dtypes.float8_e4m3fn if USE_FP8 else bf16
    # Permute W_qkv columns from per-head [q|k|v] interleave to [Q|K|V]
    # blocks (V pre-scaled by 1/sqrt(dh)), convert to fp8.
    w_blocks = W_qkv.reshape(D, H, 3, DH).transpose(0, 2, 1, 3).copy()
    w_blocks[:, 2] *= SCALE
    w_perm = w_blocks.reshape(D, F)
    # g-major layout [p, g, kt, 512]
    wq_g = np.ascontiguousarray(
        w_perm.reshape(KD, P, 3, D).transpose(1, 2, 0, 3)).astype(f8)
    wo_shuf = np.ascontiguousarray(
        W_o.reshape(KD, P, D).transpose(1, 0, 2)).astype(bf16)

    apply_gb = not (np.all(gamma == 1.0) and np.all(beta == 0.0))
    nc = _get_nc(apply_gb)
    in_maps = []
    for b in range(8):
        hb = h[:, b, :]
        in_maps.append({
            "h": np.ascontiguousarray(
                hb.reshape(NLT, P, D).transpose(1, 0, 2)).astype(bf16),
            "hT": np.ascontiguousarray(
                hb.reshape(NLT, P, KD, P).transpose(3, 0, 2, 1)).astype(f8),
            "W_qkv": wq_g,
            "W_o": wo_shuf,
            "gamma": gamma,
            "beta": beta,
        })
    res = run_bass_kernel_spmd(nc, in_maps, core_ids=list(range(8)), trace=trace)
    LAST_RESULT = res
    return np.stack(
        [res.results[b]["out"].astype(np.float32) for b in range(8)], axis=1)

